# revision 20
# baseline (speedup 1.0000x reference)
"""Trainium2 Bass kernel for span-attention pooling.

Problem shapes (hardcoded):
  x: [B=2, T=512, E=1024] f32, W: [1024, 1] f32, b: [1] f32,
  start/end: [S=2048] i32.  Output: [B, S, E] f32.

Math: out[b,s,:] = sum_{t=start[s]}^{end[s]} q[b,t] * x[b,t,:] / sum q[b,t]
with q = exp(relu(x @ W + b)).  (Equivalent to the reference's per-span
softmax over head scores, since spans are contiguous token ranges and
clamped/invalid positions carry zero weight.)

Sharding: 8 cores = (batch b in {0,1}) x (span quarter of 512 spans).
Each core computes out[b, q*512:(q+1)*512, :].  The small tensors
(W, b, start, end) are host-replicated across the 128 partitions so the
kernel needs no on-chip broadcasts (gpsimd custom ops pay a ~10us
library-load penalty on this runtime).
"""

import numpy as np

import concourse.bass as bass
import concourse.tile as tile
from concourse import bacc, mybir
from concourse import bass_utils

B, T, E = 2, 512, 1024
S, A = 2048, 30
N_CORES = 8
SQ = S // 4  # spans per core
TCH = T // 128  # T chunks of 128 partitions
SCH = SQ // 128  # span chunks of 128 partitions

F32 = mybir.dt.float32
F32R = mybir.dt.float32r
I32 = mybir.dt.int32

# dtype for the main matmul operands. float32r runs the PE at ~2x the
# fp32 rate with ~tf32 precision (1.6e-4 rel, HW-measured). The walrus
# verifier requires f32r matmul inputs to be *produced* as f32r, so the
# tiles feeding the matmul are typed MM_DT and bitcast to f32 for DVE.
MM_DT = F32R


def _f32view(ap):
    if MM_DT == F32:
        return ap
    return ap.bitcast(F32)


def _build_body(tc, out_d, x_d, wb_d, misc_d):
    nc = tc.nc
    AF = mybir.ActivationFunctionType
    OP = mybir.AluOpType
    MISC_W = 1 + TCH + 2 * SQ  # bb | tcol | stb | enb packed columns

    with (
        tc.tile_pool(name="main", bufs=1) as mainp,
        tc.tile_pool(name="outp", bufs=2) as outp,
        tc.tile_pool(name="psum", bufs=1, space="PSUM") as psp,
        tc.tile_pool(name="scr", bufs=1) as scrp,
    ):
        # x chunks on the Sync DMA ring (chunk 0 first: the DVE head
        # pipeline starts as soon as it lands).
        xts = []
        for i in range(TCH):
            xt = mainp.tile([128, E], MM_DT, tag=f"xt{i}")
            nc.sync.dma_start(xt[:], x_d[128 * i : 128 * (i + 1), :])
            xts.append(xt)

        # Replicated small tensors on the Scalar HWDGE ring (parallel
        # with x): wb first (gates the head chain), then one packed DMA
        # for bias/tcol/start/end. Few DMAs -> no completion-semaphore
        # lane reuse stalls (8 lanes total).
        wb = mainp.tile([128, E], F32)
        nc.scalar.dma_start(wb[:], wb_d[:])
        misc = mainp.tile([128, MISC_W], F32)
        nc.scalar.dma_start(misc[:], misc_d[:])
        bb = misc[:, 0:1]
        tcol = misc[:, 1 : 1 + TCH]
        stb_i = misc[:, 1 + TCH : 1 + TCH + SQ].bitcast(I32)
        enb_i = misc[:, 1 + TCH + SQ : MISC_W].bitcast(I32)

        # ones tile: Z-matmul rhs (first 64 cols) + PE warm-up rhs.
        # (f32r matmuls reject tiny moving dims; memset can't write f32r
        # -> memset f32 then copy.)
        ones_f = mainp.tile([128, 512], F32)
        nc.vector.memset(ones_f[:], 1.0)
        ones_r = mainp.tile([128, 512], MM_DT)
        nc.vector.tensor_copy(ones_r[:], ones_f[:])

        # PE warm-up: the HAM clock gate keeps an idle PE at 1.2 GHz and
        # takes ~3.4us of sustained activity to release. Dummy matmuls
        # into a scratch PSUM bank bridge the gap until the first real
        # matmul so the real ones run at 2.4 GHz.
        warm = psp.tile([128, 512], F32, name="warm", tag="warm")
        for _ in range(20):
            nc.tensor.matmul(
                warm[:], ones_r[:, 0:128], ones_r[:], start=True, stop=True
            )

        # int -> float casts (values < 2^24, exact).
        stb = mainp.tile([128, SQ], F32)
        nc.vector.tensor_copy(stb[:], stb_i[:])
        enb = mainp.tile([128, SQ], F32)
        nc.vector.tensor_copy(enb[:], enb_i[:])

        # Per T-chunk: head score h, q = exp(relu(h + b)), and
        # maskq[t, s] = (start[s] <= t <= end[s]) * q[t]  (transposed layout).
        q_col = mainp.tile([128, TCH], F32)
        rh = mainp.tile([128, TCH], F32)
        h = mainp.tile([128, TCH], F32)
        scr = scrp.tile([128, E], F32)
        m2 = scrp.tile([128, SQ], F32)
        mqs = []
        for i in range(TCH):
            # h = sum_e x[t, e] * W[e]
            nc.vector.scalar_tensor_tensor(
                scr[:],
                _f32view(xts[i][:]),
                1.0,
                wb[:],
                op0=OP.mult,
                op1=OP.mult,
                accum_out=h[:, i : i + 1],
            )
            # q = exp(relu(h + b))  (two ScalarE ops, bias folded in)
            nc.scalar.activation(rh[:, i : i + 1], h[:, i : i + 1], AF.Relu, bias=bb)
            nc.scalar.activation(q_col[:, i : i + 1], rh[:, i : i + 1], AF.Exp)
            # m2 = (end >= t) * q
            nc.vector.tensor_scalar(
                m2[:],
                enb[:],
                tcol[:, i : i + 1],
                q_col[:, i : i + 1],
                op0=OP.is_ge,
                op1=OP.mult,
            )
            # maskq = (start <= t) * m2
            mq = mainp.tile([128, SQ], MM_DT, tag=f"mq{i}")
            nc.vector.scalar_tensor_tensor(
                mq[:],
                stb[:],
                tcol[:, i : i + 1],
                m2[:],
                op0=OP.is_le,
                op1=OP.mult,
            )
            mqs.append(mq)

        # Matmuls, i-major in two passes of two span-chunks (PSUM: 3
        # banks per span-chunk x 2 in flight = 6 of 8 banks):
        #   out_psum[s, e] = sum_t maskq[t, s] * x[t, e]
        #   Z[s]           = sum_t maskq[t, s]
        for jj in (0, 2):
            pos = {}
            for j in (jj, jj + 1):
                pos[j] = (
                    psp.tile([128, 512], F32, name=f"po0_{j}", tag=f"po0_{j % 2}"),
                    psp.tile([128, 512], F32, name=f"po1_{j}", tag=f"po1_{j % 2}"),
                    psp.tile([128, 64], F32, name=f"zp_{j}", tag=f"zp_{j % 2}"),
                )
            for i in range(TCH):
                st_, sp_ = (i == 0), (i == TCH - 1)
                for j in (jj, jj + 1):
                    lhsT = mqs[i][:, 128 * j : 128 * (j + 1)]
                    po0, po1, zp = pos[j]
                    nc.tensor.matmul(po0[:], lhsT, xts[i][:, 0:512], start=st_, stop=sp_)
                    nc.tensor.matmul(po1[:], lhsT, xts[i][:, 512:1024], start=st_, stop=sp_)
                    nc.tensor.matmul(zp[:], lhsT, ones_r[:, 0:64], start=st_, stop=sp_)
            for j in (jj, jj + 1):
                po0, po1, zp = pos[j]
                rz = scrp.tile([128, 1], F32, tag=f"rz{j % 2}")
                nc.vector.reciprocal(rz[:], zp[:, 0:1])
                ob = outp.tile([128, E], F32)
                # normalization split across ScalarE and VectorE; output
                # DMAs alternate rings so issue overlaps.
                nc.scalar.mul(ob[:, 0:512], po0[:], rz[:])
                nc.vector.tensor_scalar_mul(ob[:, 512:1024], po1[:], rz[:])
                dma_eng = nc.sync if j % 2 == 0 else nc.scalar
                dma_eng.dma_start(out_d[128 * j : 128 * (j + 1), :], ob[:])


def build_kernel():
    nc = bacc.Bacc(
        "TRN2",
        target_bir_lowering=False,
        debug=False,
        num_devices=N_CORES,
    )
    # x is declared MM_DT (same 4-byte layout as f32) so the HWDGE load
    # into the f32r-typed xt tiles is cast-free and verifier-consistent.
    MISC_W = 1 + TCH + 2 * SQ
    x_d = nc.dram_tensor("x", [T, E], MM_DT, kind="ExternalInput").ap()
    wb_d = nc.dram_tensor("wb", [128, E], F32, kind="ExternalInput").ap()
    misc_d = nc.dram_tensor("misc", [128, MISC_W], F32, kind="ExternalInput").ap()
    out_d = nc.dram_tensor("out", [SQ, E], F32, kind="ExternalOutput").ap()

    with tile.TileContext(nc) as tc:
        _build_body(tc, out_d, x_d, wb_d, misc_d)
    nc.compile()
    return nc


_NC_CACHE = None


def _get_nc():
    global _NC_CACHE
    if _NC_CACHE is None:
        _NC_CACHE = build_kernel()
    return _NC_CACHE


def _make_in_maps(x, W, b, start, end):
    x = np.asarray(x, dtype=np.float32)
    start = np.asarray(start, dtype=np.int32)
    end = np.asarray(end, dtype=np.int32)
    wb = np.ascontiguousarray(
        np.broadcast_to(np.asarray(W, np.float32).reshape(1, E), (128, E))
    )
    tcol = (
        np.arange(128, dtype=np.float32)[:, None]
        + 128.0 * np.arange(TCH, dtype=np.float32)[None, :]
    ).astype(np.float32)
    in_maps = []
    for core in range(N_CORES):
        bb_idx, qq = divmod(core, 4)
        st_q = start[qq * SQ : (qq + 1) * SQ]
        en_q = end[qq * SQ : (qq + 1) * SQ]
        # packed misc: [b | tcol | start(int32 bits) | end(int32 bits)]
        misc = np.empty((128, 1 + TCH + 2 * SQ), np.float32)
        misc[:, 0] = np.float32(np.asarray(b, np.float32).reshape(1)[0])
        misc[:, 1 : 1 + TCH] = tcol
        misc[:, 1 + TCH : 1 + TCH + SQ] = np.broadcast_to(
            st_q.view(np.float32)[None, :], (128, SQ)
        )
        misc[:, 1 + TCH + SQ :] = np.broadcast_to(
            en_q.view(np.float32)[None, :], (128, SQ)
        )
        in_maps.append(
            {
                "x": np.ascontiguousarray(x[bb_idx]),
                "wb": wb,
                "misc": np.ascontiguousarray(misc),
            }
        )
    return in_maps


def run(x, W, b, start, end, trace=False, trace_cores=None):
    """Run on 8 cores; returns (out[B,S,E] f32, BassKernelResults)."""
    nc = _get_nc()
    in_maps = _make_in_maps(x, W, b, start, end)
    res = bass_utils.run_bass_kernel_spmd(
        nc,
        in_maps,
        core_ids=list(range(N_CORES)),
        trace=trace,
        trace_cores=trace_cores,
    )
    out = np.empty((B, S, E), np.float32)
    for core in range(N_CORES):
        bb_idx, qq = divmod(core, 4)
        out[bb_idx, qq * SQ : (qq + 1) * SQ] = res.results[core]["out"]
    return out, res


def kernel(x, W, b, start, end):
    out, _ = run(x, W, b, start, end, trace=False)
    return out


# revision 26
# speedup vs baseline: 1.0016x; 1.0016x over previous
"""Trainium2 Bass kernel for span-attention pooling.

Problem shapes (hardcoded):
  x: [B=2, T=512, E=1024] f32, W: [1024, 1] f32, b: [1] f32,
  start/end: [S=2048] i32.  Output: [B, S, E] f32.

Math: out[b,s,:] = sum_{t=start[s]}^{end[s]} q[b,t] * x[b,t,:] / sum q[b,t]
with q = exp(relu(x @ W + b)).  (Equivalent to the reference's per-span
softmax over head scores, since spans are contiguous token ranges and
clamped/invalid positions carry zero weight.)

Sharding: 8 cores = (batch b in {0,1}) x (span quarter of 512 spans).
Each core computes out[b, q*512:(q+1)*512, :].

Internals run in fp16: x and the mask weights are fp16 (PE matmul
accumulates in fp32; DVE gets its 2x/4x packed modes), start/end/t
values <= 511 are exact in fp16, and the per-token softmax scale q is
applied identically to numerator and denominator so its rounding
cancels. HW-measured absmax-relative error ~= 6e-4.
"""

import numpy as np

import concourse.bass as bass
import concourse.tile as tile
from concourse import bacc, mybir
from concourse import bass_utils

B, T, E = 2, 512, 1024
S, A = 2048, 30
N_CORES = 8
SQ = S // 4  # spans per core
TCH = T // 128  # T chunks of 128 partitions
SCH = SQ // 128  # span chunks of 128 partitions

F32 = mybir.dt.float32
F16 = mybir.dt.float16
I32 = mybir.dt.int32

# packed misc columns (fp16): [b f32 bits | tcol f32 bits | stb | enb]
MISC_W = 2 + 2 * TCH + 2 * SQ
OFF_TCOL = 2
OFF_STB = 2 + 2 * TCH
OFF_ENB = OFF_STB + SQ


def _build_body(tc, out_d, x_d, w_d, misc_d):
    nc = tc.nc
    AF = mybir.ActivationFunctionType
    OP = mybir.AluOpType

    with (
        tc.tile_pool(name="main", bufs=1) as mainp,
        tc.tile_pool(name="outp", bufs=2) as outp,
        tc.tile_pool(name="psum", bufs=1, space="PSUM") as psp,
        tc.tile_pool(name="scr", bufs=1) as scrp,
    ):
        # x chunks: SWDGE (gpsimd) DMA with f32 -> fp16 cast, chunk 0
        # first (it gates the head-score pipeline).
        xts = []
        for i in range(TCH):
            xt = mainp.tile([128, E], F16, tag=f"xt{i}")
            nc.gpsimd.dma_start(xt[:], x_d[128 * i : 128 * (i + 1), :])
            xts.append(xt)

        # W row (tiny, casts to fp16 in flight) + packed misc on the two
        # HWDGE rings, parallel with x.
        w_row = mainp.tile([1, E], F16)
        nc.gpsimd.dma_start(w_row[0:1, :], w_d.rearrange("(o e) -> o e", o=1))
        misc = mainp.tile([128, MISC_W], F16)
        nc.sync.dma_start(misc[:], misc_d[:])
        bb = misc[:, 0:2].bitcast(F32)  # [128, 1]
        tcol = misc[:, OFF_TCOL:OFF_STB].bitcast(F32)  # [128, TCH] f32
        stb = misc[:, OFF_STB:OFF_ENB]  # [128, SQ] fp16
        enb = misc[:, OFF_ENB:MISC_W]  # [128, SQ] fp16

        # ones tile: Z-matmul rhs (first 64 cols), PE warm-up operand,
        # and the [1,128] slice is the broadcast lhsT for W.
        ones16 = mainp.tile([128, 512], F16)
        nc.vector.memset(ones16[:], 1.0)

        # Broadcast W across partitions on the PE (ones[1,128].T @ w_row)
        # -- avoids DMAing a 128x-replicated copy of W through HBM.
        wbp = psp.tile([128, 512], F32, name="wbp", tag="p4")
        wb16 = mainp.tile([128, E], F16)
        for nn_ in range(2):
            nc.tensor.matmul(
                wbp[:],
                ones16[0:1, 0:128],
                w_row[0:1, 512 * nn_ : 512 * (nn_ + 1)],
                start=True,
                stop=True,
            )
            nc.scalar.copy(wb16[:, 512 * nn_ : 512 * (nn_ + 1)], wbp[:])

        # PE warm-up: the HAM clock gate keeps an idle PE at 1.2 GHz and
        # takes ~3.4us of sustained activity to release. Dummy matmuls
        # into a scratch PSUM bank bridge the gap until the first real
        # matmul so the real ones run at 2.4 GHz.
        warm = psp.tile([128, 512], F32, name="warm", tag="p4")
        for _ in range(20):
            nc.tensor.matmul(
                warm[:], ones16[:, 0:128], ones16[:], start=True, stop=True
            )

        # Per T-chunk: head score h, q = exp(relu(h + b)), and
        # maskq[t, s] = (start[s] <= t <= end[s]) * q[t]  (transposed layout).
        q_col = mainp.tile([128, TCH], F32)
        rh = mainp.tile([128, TCH], F32)
        h = mainp.tile([128, TCH], F32)
        scr = scrp.tile([128, E], F16)
        m2 = scrp.tile([128, SQ], F16)
        mqs = []
        for i in range(TCH):
            # h = sum_e x[t, e] * W[e]
            nc.vector.scalar_tensor_tensor(
                scr[:],
                xts[i][:],
                1.0,
                wb16[:],
                op0=OP.mult,
                op1=OP.mult,
                accum_out=h[:, i : i + 1],
            )
            # q = exp(relu(h + b))  (two ScalarE ops, bias folded in)
            nc.scalar.activation(
                rh[:, i : i + 1], h[:, i : i + 1], AF.Relu, bias=bb
            )
            nc.scalar.activation(q_col[:, i : i + 1], rh[:, i : i + 1], AF.Exp)
            # m2 = (end >= t) * q
            nc.vector.tensor_scalar(
                m2[:],
                enb[:],
                tcol[:, i : i + 1],
                q_col[:, i : i + 1],
                op0=OP.is_ge,
                op1=OP.mult,
            )
            # maskq = (start <= t) * m2
            mq = mainp.tile([128, SQ], F16, tag=f"mq{i}")
            nc.vector.scalar_tensor_tensor(
                mq[:],
                stb[:],
                tcol[:, i : i + 1],
                m2[:],
                op0=OP.is_le,
                op1=OP.mult,
            )
            mqs.append(mq)

        # Matmuls in two phases so PSUM fits while all four span-chunks
        # progress i-major (each mq_i is consumed as soon as it lands):
        #   phase 1: out[:, 0:512] for all j + Z (all four Z live in one
        #            PSUM bank at different column offsets)
        #   phase 2: out[:, 512:1024] for all j
        poA = [psp.tile([128, 512], F32, name=f"poA{j}", tag=f"p{j}") for j in range(4)]
        # Z lives in two alternating banks: a PSUM accumulation group
        # zeroes its whole 2KB bank on start, so groups for different j
        # must not share a bank while both are open.
        zps = [
            psp.tile([128, 64], F32, name=f"zp{j}", tag=f"zp{j % 2}") for j in range(4)
        ]
        for i in range(TCH):
            st_, sp_ = (i == 0), (i == TCH - 1)
            for j in range(SCH):
                lhsT = mqs[i][:, 128 * j : 128 * (j + 1)]
                nc.tensor.matmul(poA[j][:], lhsT, xts[i][:, 0:512], start=st_, stop=sp_)
                if j < 2:
                    nc.tensor.matmul(
                        zps[j][:], lhsT, ones16[:, 0:64], start=st_, stop=sp_
                    )
        # j2/j3 Z groups run after j0/j1's are read out (bank reuse)
        for j in (2, 3):
            for i in range(TCH):
                nc.tensor.matmul(
                    zps[j][:],
                    mqs[i][:, 128 * j : 128 * (j + 1)],
                    ones16[:, 0:64],
                    start=(i == 0),
                    stop=(i == TCH - 1),
                )

        rzs = []
        for j in range(SCH):
            rzj = scrp.tile([128, 1], F32, name=f"rz{j}", tag=f"rz{j}")
            nc.vector.reciprocal(rzj[:], zps[j][:, 0:1])
            rzs.append(rzj)

        obs = [outp.tile([128, E], F32, name=f"ob{j}", tag=f"ob{j % 2}") for j in range(4)]
        for j in range(SCH):
            if j % 2 == 0:
                nc.scalar.mul(obs[j][:, 0:512], poA[j][:], rzs[j][:])
            else:
                nc.vector.tensor_scalar_mul(obs[j][:, 0:512], poA[j][:], rzs[j][:])
            dma_eng = nc.sync if j % 2 == 0 else nc.scalar
            dma_eng.dma_start(out_d[128 * j : 128 * (j + 1), 0:512], obs[j][:, 0:512])

        # phase 2 (tags: j0/j1 take the two remaining banks; j2/j3 reuse
        # the banks freed by the j0/j1 phase-1 normalizations)
        pb_tags = ["p4", "p5", "p0", "p1"]
        poB = [
            psp.tile([128, 512], F32, name=f"poB{j}", tag=pb_tags[j]) for j in range(4)
        ]
        for i in range(TCH):
            st_, sp_ = (i == 0), (i == TCH - 1)
            for j in range(SCH):
                lhsT = mqs[i][:, 128 * j : 128 * (j + 1)]
                nc.tensor.matmul(
                    poB[j][:], lhsT, xts[i][:, 512:1024], start=st_, stop=sp_
                )
        for j in range(SCH):
            if j % 2 == 0:
                nc.scalar.mul(obs[j][:, 512:1024], poB[j][:], rzs[j][:])
            else:
                nc.vector.tensor_scalar_mul(obs[j][:, 512:1024], poB[j][:], rzs[j][:])
            dma_eng = nc.sync if j % 2 == 0 else nc.scalar
            dma_eng.dma_start(
                out_d[128 * j : 128 * (j + 1), 512:1024], obs[j][:, 512:1024]
            )


def build_kernel():
    nc = bacc.Bacc(
        "TRN2",
        target_bir_lowering=False,
        debug=False,
        num_devices=N_CORES,
    )
    x_d = nc.dram_tensor("x", [T, E], F32, kind="ExternalInput").ap()
    w_d = nc.dram_tensor("w", [E], F32, kind="ExternalInput").ap()
    misc_d = nc.dram_tensor("misc", [128, MISC_W], F16, kind="ExternalInput").ap()
    out_d = nc.dram_tensor("out", [SQ, E], F32, kind="ExternalOutput").ap()

    with tile.TileContext(nc) as tc:
        _build_body(tc, out_d, x_d, w_d, misc_d)
    nc.compile()
    return nc


_NC_CACHE = None


def _get_nc():
    global _NC_CACHE
    if _NC_CACHE is None:
        _NC_CACHE = build_kernel()
    return _NC_CACHE


def _make_in_maps(x, W, b, start, end):
    x = np.asarray(x, dtype=np.float32)
    start = np.asarray(start, dtype=np.int32)
    end = np.asarray(end, dtype=np.int32)
    w_flat = np.ascontiguousarray(np.asarray(W, np.float32).reshape(E))
    b_f32 = np.asarray(b, np.float32).reshape(1)
    tcol = (
        np.arange(128, dtype=np.float32)[:, None]
        + 128.0 * np.arange(TCH, dtype=np.float32)[None, :]
    ).astype(np.float32)
    in_maps = []
    for core in range(N_CORES):
        bb_idx, qq = divmod(core, 4)
        st_q = start[qq * SQ : (qq + 1) * SQ]
        en_q = end[qq * SQ : (qq + 1) * SQ]
        misc = np.empty((128, MISC_W), np.float16)
        misc[:, 0:2] = b_f32.view(np.float16)[None, :]
        misc[:, OFF_TCOL:OFF_STB] = tcol.view(np.float16)
        # start/end values < 2^11 are exact in fp16
        misc[:, OFF_STB:OFF_ENB] = st_q.astype(np.float16)[None, :]
        misc[:, OFF_ENB:MISC_W] = en_q.astype(np.float16)[None, :]
        in_maps.append(
            {
                "x": np.ascontiguousarray(x[bb_idx]),
                "w": w_flat,
                "misc": np.ascontiguousarray(misc),
            }
        )
    return in_maps


def run(x, W, b, start, end, trace=False, trace_cores=None):
    """Run on 8 cores; returns (out[B,S,E] f32, BassKernelResults)."""
    nc = _get_nc()
    in_maps = _make_in_maps(x, W, b, start, end)
    res = bass_utils.run_bass_kernel_spmd(
        nc,
        in_maps,
        core_ids=list(range(N_CORES)),
        trace=trace,
        trace_cores=trace_cores,
    )
    out = np.empty((B, S, E), np.float32)
    for core in range(N_CORES):
        bb_idx, qq = divmod(core, 4)
        out[bb_idx, qq * SQ : (qq + 1) * SQ] = res.results[core]["out"]
    return out, res


def kernel(x, W, b, start, end):
    out, _ = run(x, W, b, start, end, trace=False)
    return out


# revision 27
# speedup vs baseline: 1.1103x; 1.1085x over previous
"""Trainium2 Bass kernel for span-attention pooling.

Problem shapes (hardcoded):
  x: [B=2, T=512, E=1024] f32, W: [1024, 1] f32, b: [1] f32,
  start/end: [S=2048] i32.  Output: [B, S, E] f32.

Math: out[b,s,:] = sum_{t=start[s]}^{end[s]} q[b,t] * x[b,t,:] / sum q[b,t]
with q = exp(relu(x @ W + b)).  (Equivalent to the reference's per-span
softmax over head scores, since spans are contiguous token ranges and
clamped/invalid positions carry zero weight.)

Sharding: 8 cores = (batch b in {0,1}) x (span quarter of 512 spans).
Each core computes out[b, q*512:(q+1)*512, :].

Internals run in fp16: x and the mask weights are fp16 (PE matmul
accumulates in fp32; DVE gets its 2x/4x packed modes), start/end/t
values <= 511 are exact in fp16, and the per-token softmax scale q is
applied identically to numerator and denominator so its rounding
cancels. HW-measured absmax-relative error ~= 6e-4.
"""

import numpy as np

import concourse.bass as bass
import concourse.tile as tile
from concourse import bacc, mybir
from concourse import bass_utils

B, T, E = 2, 512, 1024
S, A = 2048, 30
N_CORES = 8
SQ = S // 4  # spans per core
TCH = T // 128  # T chunks of 128 partitions
SCH = SQ // 128  # span chunks of 128 partitions

F32 = mybir.dt.float32
F16 = mybir.dt.float16
I32 = mybir.dt.int32

# packed misc columns (fp16): [b f32 bits | tcol f32 bits | stb | enb]
MISC_W = 2 + 2 * TCH + 2 * SQ
OFF_TCOL = 2
OFF_STB = 2 + 2 * TCH
OFF_ENB = OFF_STB + SQ


def _build_body(tc, out_d, x_d, w_d, misc_d):
    nc = tc.nc
    AF = mybir.ActivationFunctionType
    OP = mybir.AluOpType

    with (
        tc.tile_pool(name="main", bufs=1) as mainp,
        tc.tile_pool(name="outp", bufs=2) as outp,
        tc.tile_pool(name="psum", bufs=1, space="PSUM") as psp,
        tc.tile_pool(name="scr", bufs=1) as scrp,
    ):
        # x chunks: SWDGE (gpsimd) DMA with f32 -> fp16 cast, chunk 0
        # first (it gates the head-score pipeline).
        xts = []
        for i in range(TCH):
            xt = mainp.tile([128, E], F16, tag=f"xt{i}")
            nc.gpsimd.dma_start(xt[:], x_d[128 * i : 128 * (i + 1), :])
            xts.append(xt)

        # Replicated fp16 W on the Scalar HWDGE ring, packed misc on the
        # Sync ring -- all three DMA paths (gpsimd/sync/scalar) run in
        # parallel.
        wb16 = mainp.tile([128, E], F16)
        nc.scalar.dma_start(wb16[:], w_d[:])
        misc = mainp.tile([128, MISC_W], F16)
        nc.sync.dma_start(misc[:], misc_d[:])
        bb = misc[:, 0:2].bitcast(F32)  # [128, 1]
        tcol = misc[:, OFF_TCOL:OFF_STB].bitcast(F32)  # [128, TCH] f32
        stb = misc[:, OFF_STB:OFF_ENB]  # [128, SQ] fp16
        enb = misc[:, OFF_ENB:MISC_W]  # [128, SQ] fp16

        # ones tile: Z-matmul rhs (first 64 cols), PE warm-up operand,
        # and the [1,128] slice is the broadcast lhsT for W.
        ones16 = mainp.tile([128, 512], F16)
        nc.vector.memset(ones16[:], 1.0)

        # PE warm-up: the HAM clock gate keeps an idle PE at 1.2 GHz and
        # takes ~3.4us of sustained activity to release. Dummy matmuls
        # into a scratch PSUM bank bridge the gap until the first real
        # matmul so the real ones run at 2.4 GHz.
        warm = psp.tile([128, 512], F32, name="warm", tag="p5")
        for _ in range(12):
            nc.tensor.matmul(
                warm[:], ones16[:, 0:128], ones16[:], start=True, stop=True
            )

        # Per T-chunk: head score h, q = exp(relu(h + b)), and
        # maskq[t, s] = (start[s] <= t <= end[s]) * q[t]  (transposed layout).
        q_col = mainp.tile([128, TCH], F32)
        rh = mainp.tile([128, TCH], F32)
        h = mainp.tile([128, TCH], F32)
        scr = scrp.tile([128, E], F16)
        m2 = scrp.tile([128, SQ], F16)
        mqs = []
        for i in range(TCH):
            # h = sum_e x[t, e] * W[e]
            nc.vector.scalar_tensor_tensor(
                scr[:],
                xts[i][:],
                1.0,
                wb16[:],
                op0=OP.mult,
                op1=OP.mult,
                accum_out=h[:, i : i + 1],
            )
            # q = exp(relu(h + b))  (two ScalarE ops, bias folded in)
            nc.scalar.activation(
                rh[:, i : i + 1], h[:, i : i + 1], AF.Relu, bias=bb
            )
            nc.scalar.activation(q_col[:, i : i + 1], rh[:, i : i + 1], AF.Exp)
            # m2 = (end >= t) * q
            nc.vector.tensor_scalar(
                m2[:],
                enb[:],
                tcol[:, i : i + 1],
                q_col[:, i : i + 1],
                op0=OP.is_ge,
                op1=OP.mult,
            )
            # maskq = (start <= t) * m2
            mq = mainp.tile([128, SQ], F16, tag=f"mq{i}")
            nc.vector.scalar_tensor_tensor(
                mq[:],
                stb[:],
                tcol[:, i : i + 1],
                m2[:],
                op0=OP.is_le,
                op1=OP.mult,
            )
            mqs.append(mq)

        # Matmuls in two phases so PSUM fits while all four span-chunks
        # progress i-major (each mq_i is consumed as soon as it lands):
        #   phase 1: out[:, 0:512] for all j + Z (all four Z live in one
        #            PSUM bank at different column offsets)
        #   phase 2: out[:, 512:1024] for all j
        poA = [psp.tile([128, 512], F32, name=f"poA{j}", tag=f"p{j}") for j in range(4)]
        # Z lives in two alternating banks: a PSUM accumulation group
        # zeroes its whole 2KB bank on start, so groups for different j
        # must not share a bank while both are open.
        zps = [
            psp.tile([128, 64], F32, name=f"zp{j}", tag=f"zp{j % 2}") for j in range(4)
        ]
        pb_tags = ["p4", "p5", "p0", "p1"]
        poB = [
            psp.tile([128, 512], F32, name=f"poB{j}", tag=pb_tags[j]) for j in range(4)
        ]
        for i in range(TCH):
            st_, sp_ = (i == 0), (i == TCH - 1)
            for j in range(SCH):
                lhsT = mqs[i][:, 128 * j : 128 * (j + 1)]
                nc.tensor.matmul(poA[j][:], lhsT, xts[i][:, 0:512], start=st_, stop=sp_)
                if j < 2:
                    nc.tensor.matmul(
                        zps[j][:], lhsT, ones16[:, 0:64], start=st_, stop=sp_
                    )
                    nc.tensor.matmul(
                        poB[j][:], lhsT, xts[i][:, 512:1024], start=st_, stop=sp_
                    )
        # j2/j3 Z groups + second-half matmuls run as the j0/j1 banks free up
        for j in (2, 3):
            for i in range(TCH):
                lhsT = mqs[i][:, 128 * j : 128 * (j + 1)]
                nc.tensor.matmul(
                    zps[j][:], lhsT, ones16[:, 0:64],
                    start=(i == 0), stop=(i == TCH - 1),
                )
                nc.tensor.matmul(
                    poB[j][:], lhsT, xts[i][:, 512:1024],
                    start=(i == 0), stop=(i == TCH - 1),
                )

        rzs = []
        for j in range(SCH):
            rzj = scrp.tile([128, 1], F32, name=f"rz{j}", tag=f"rz{j}")
            nc.vector.reciprocal(rzj[:], zps[j][:, 0:1])
            rzs.append(rzj)

        obs = [outp.tile([128, E], F32, name=f"ob{j}", tag=f"ob{j % 2}") for j in range(4)]
        for j in range(SCH):
            if j % 2 == 0:
                nc.scalar.mul(obs[j][:, 0:512], poA[j][:], rzs[j][:])
            else:
                nc.vector.tensor_scalar_mul(obs[j][:, 0:512], poA[j][:], rzs[j][:])
            dma_eng = nc.sync if j % 2 == 0 else nc.scalar
            dma_eng.dma_start(out_d[128 * j : 128 * (j + 1), 0:512], obs[j][:, 0:512])

        for j in range(SCH):
            if j % 2 == 0:
                nc.scalar.mul(obs[j][:, 512:1024], poB[j][:], rzs[j][:])
            else:
                nc.vector.tensor_scalar_mul(obs[j][:, 512:1024], poB[j][:], rzs[j][:])
            dma_eng = nc.sync if j % 2 == 0 else nc.scalar
            dma_eng.dma_start(
                out_d[128 * j : 128 * (j + 1), 512:1024], obs[j][:, 512:1024]
            )


def build_kernel():
    nc = bacc.Bacc(
        "TRN2",
        target_bir_lowering=False,
        debug=False,
        num_devices=N_CORES,
    )
    x_d = nc.dram_tensor("x", [T, E], F32, kind="ExternalInput").ap()
    w_d = nc.dram_tensor("w", [128, E], F16, kind="ExternalInput").ap()
    misc_d = nc.dram_tensor("misc", [128, MISC_W], F16, kind="ExternalInput").ap()
    out_d = nc.dram_tensor("out", [SQ, E], F32, kind="ExternalOutput").ap()

    with tile.TileContext(nc) as tc:
        _build_body(tc, out_d, x_d, w_d, misc_d)
    nc.compile()
    return nc


_NC_CACHE = None


def _get_nc():
    global _NC_CACHE
    if _NC_CACHE is None:
        _NC_CACHE = build_kernel()
    return _NC_CACHE


def _make_in_maps(x, W, b, start, end):
    x = np.asarray(x, dtype=np.float32)
    start = np.asarray(start, dtype=np.int32)
    end = np.asarray(end, dtype=np.int32)
    w16 = np.ascontiguousarray(
        np.broadcast_to(np.asarray(W, np.float32).reshape(1, E).astype(np.float16), (128, E))
    )
    b_f32 = np.asarray(b, np.float32).reshape(1)
    tcol = (
        np.arange(128, dtype=np.float32)[:, None]
        + 128.0 * np.arange(TCH, dtype=np.float32)[None, :]
    ).astype(np.float32)
    in_maps = []
    for core in range(N_CORES):
        bb_idx, qq = divmod(core, 4)
        st_q = start[qq * SQ : (qq + 1) * SQ]
        en_q = end[qq * SQ : (qq + 1) * SQ]
        misc = np.empty((128, MISC_W), np.float16)
        misc[:, 0:2] = b_f32.view(np.float16)[None, :]
        misc[:, OFF_TCOL:OFF_STB] = tcol.view(np.float16)
        # start/end values < 2^11 are exact in fp16
        misc[:, OFF_STB:OFF_ENB] = st_q.astype(np.float16)[None, :]
        misc[:, OFF_ENB:MISC_W] = en_q.astype(np.float16)[None, :]
        in_maps.append(
            {
                "x": np.ascontiguousarray(x[bb_idx]),
                "w": w16,
                "misc": np.ascontiguousarray(misc),
            }
        )
    return in_maps


def run(x, W, b, start, end, trace=False, trace_cores=None):
    """Run on 8 cores; returns (out[B,S,E] f32, BassKernelResults)."""
    nc = _get_nc()
    in_maps = _make_in_maps(x, W, b, start, end)
    res = bass_utils.run_bass_kernel_spmd(
        nc,
        in_maps,
        core_ids=list(range(N_CORES)),
        trace=trace,
        trace_cores=trace_cores,
    )
    out = np.empty((B, S, E), np.float32)
    for core in range(N_CORES):
        bb_idx, qq = divmod(core, 4)
        out[bb_idx, qq * SQ : (qq + 1) * SQ] = res.results[core]["out"]
    return out, res


def kernel(x, W, b, start, end):
    out, _ = run(x, W, b, start, end, trace=False)
    return out


# revision 30
# speedup vs baseline: 1.3726x; 1.2362x over previous
"""Trainium2 Bass kernel for span-attention pooling.

Problem shapes (hardcoded):
  x: [B=2, T=512, E=1024] f32, W: [1024, 1] f32, b: [1] f32,
  start/end: [S=2048] i32.  Output: [B, S, E] f32.

Math: out[b,s,:] = sum_{t=start[s]}^{end[s]} q[b,t] * x[b,t,:] / sum q[b,t]
with q = exp(relu(x @ W + b)).  (Equivalent to the reference's per-span
softmax over head scores, since spans are contiguous token ranges and
clamped/invalid positions carry zero weight.)

Sharding: 8 cores = (batch b in {0,1}) x (512-span group). Spans are
sorted by start on the host and split into quarters; since a span
covers at most 30 consecutive tokens, each quarter's spans live inside
a window of <= 256 tokens, so each core only loads and contracts its
256-token x slice (K=256 instead of 512). If an exotic span
distribution breaks the window property, the kernel falls back to
unsorted quarters with the full K=512.

Internals run in fp16 (PE accumulates in fp32; start/end/t values are
exact in fp16; the per-token softmax scale q hits numerator and
denominator identically so its rounding largely cancels). HW-measured
absmax-relative error ~4e-4.
"""

import numpy as np

import concourse.bass as bass
import concourse.tile as tile
from concourse import bacc, mybir
from concourse import bass_utils

B, T, E = 2, 512, 1024
S, A = 2048, 30
N_CORES = 8
SQ = S // 4  # spans per core
SCH = SQ // 128  # span chunks of 128 partitions

F32 = mybir.dt.float32
F16 = mybir.dt.float16
I32 = mybir.dt.int32


def _misc_layout(tch):
    # packed misc columns (fp16): [b f32 bits | tcol f32 bits | stb | enb]
    off_tcol = 2
    off_stb = off_tcol + 2 * tch
    off_enb = off_stb + SQ
    return off_tcol, off_stb, off_enb, off_enb + SQ


def _build_body(tc, tch, out_d, x_d, w_d, misc_d):
    nc = tc.nc
    AF = mybir.ActivationFunctionType
    OP = mybir.AluOpType
    OFF_TCOL, OFF_STB, OFF_ENB, MISC_W = _misc_layout(tch)

    with (
        tc.tile_pool(name="main", bufs=1) as mainp,
        tc.tile_pool(name="outp", bufs=2) as outp,
        tc.tile_pool(name="psum", bufs=1, space="PSUM") as psp,
        tc.tile_pool(name="scr", bufs=1) as scrp,
    ):
        # x chunks (fp16) on the Sync HWDGE ring, chunk 0 first (it
        # gates the head-score pipeline).
        xts = []
        for i in range(tch):
            xt = mainp.tile([128, E], F16, name=f"xt{i}", tag=f"xt{i}")
            nc.sync.dma_start(xt[:], x_d[128 * i : 128 * (i + 1), :])
            xts.append(xt)

        # Replicated fp16 W + packed misc on the Scalar HWDGE ring.
        wb16 = mainp.tile([128, E], F16)
        nc.scalar.dma_start(wb16[:], w_d[:])
        misc = mainp.tile([128, MISC_W], F16)
        nc.scalar.dma_start(misc[:], misc_d[:])
        bb = misc[:, 0:2].bitcast(F32)
        tcol = misc[:, OFF_TCOL:OFF_STB].bitcast(F32)
        stb = misc[:, OFF_STB:OFF_ENB]
        enb = misc[:, OFF_ENB:MISC_W]

        # ones tile: Z-matmul rhs (first 64 cols) + PE warm-up operand.
        ones16 = mainp.tile([128, 512], F16)
        nc.vector.memset(ones16[:], 1.0)

        # PE warm-up: the HAM clock gate keeps an idle PE at 1.2 GHz and
        # takes ~3.4us of sustained activity to release; dummy matmuls
        # bridge the gap so the real ones run at 2.4 GHz.
        warm = psp.tile([128, 512], F32, name="warm", tag="p5")
        for _ in range(10):
            nc.tensor.matmul(
                warm[:], ones16[:, 0:128], ones16[:], start=True, stop=True
            )

        # Per token-chunk: head score h, q = exp(relu(h + b)), and
        # maskq[t, s] = (start[s] <= t <= end[s]) * q[t]  (transposed layout).
        q_col = mainp.tile([128, tch], F32)
        rh = mainp.tile([128, tch], F32)
        h = mainp.tile([128, tch], F32)
        scr = scrp.tile([128, E], F16)
        m2 = scrp.tile([128, SQ], F16)
        mqs = []
        for i in range(tch):
            # h = sum_e x[t, e] * W[e]
            nc.vector.scalar_tensor_tensor(
                scr[:],
                xts[i][:],
                1.0,
                wb16[:],
                op0=OP.mult,
                op1=OP.mult,
                accum_out=h[:, i : i + 1],
            )
            # mask ops at high priority: finishing chunk i's mask
            # (which unblocks the PE) beats starting chunk i+1's head.
            with tc.high_priority():
                nc.scalar.activation(
                    rh[:, i : i + 1], h[:, i : i + 1], AF.Relu, bias=bb
                )
                nc.scalar.activation(q_col[:, i : i + 1], rh[:, i : i + 1], AF.Exp)
                # m2 = (end >= t) * q
                nc.vector.tensor_scalar(
                    m2[:],
                    enb[:],
                    tcol[:, i : i + 1],
                    q_col[:, i : i + 1],
                    op0=OP.is_ge,
                    op1=OP.mult,
                )
                # maskq = (start <= t) * m2
                mq = mainp.tile([128, SQ], F16, name=f"mq{i}", tag=f"mq{i}")
                nc.vector.scalar_tensor_tensor(
                    mq[:],
                    stb[:],
                    tcol[:, i : i + 1],
                    m2[:],
                    op0=OP.is_le,
                    op1=OP.mult,
                )
            mqs.append(mq)

        # Matmuls: 8 PSUM banks hold poA (first E half) for all four
        # span-chunks, Z for j0/j1, and poB (second half) for j0/j1.
        # j2/j3's Z and poB groups run as those banks free up.
        #   po[s, e] = sum_t maskq[t, s] * x[t, e];  Z[s] = sum_t maskq[t, s]
        poA = [psp.tile([128, 512], F32, name=f"poA{j}", tag=f"p{j}") for j in range(4)]
        zps = [
            psp.tile([128, 64], F32, name=f"zp{j}", tag=f"zp{j % 2}") for j in range(4)
        ]
        pb_tags = ["p4", "p5", "p0", "p1"]
        poB = [
            psp.tile([128, 512], F32, name=f"poB{j}", tag=pb_tags[j]) for j in range(4)
        ]
        for i in range(tch):
            st_, sp_ = (i == 0), (i == tch - 1)
            for j in range(SCH):
                lhsT = mqs[i][:, 128 * j : 128 * (j + 1)]
                nc.tensor.matmul(poA[j][:], lhsT, xts[i][:, 0:512], start=st_, stop=sp_)
                if j < 2:
                    nc.tensor.matmul(
                        zps[j][:], lhsT, ones16[:, 0:64], start=st_, stop=sp_
                    )
                    nc.tensor.matmul(
                        poB[j][:], lhsT, xts[i][:, 512:1024], start=st_, stop=sp_
                    )
        for j in (2, 3):
            for i in range(tch):
                lhsT = mqs[i][:, 128 * j : 128 * (j + 1)]
                nc.tensor.matmul(
                    zps[j][:], lhsT, ones16[:, 0:64],
                    start=(i == 0), stop=(i == tch - 1),
                )
                nc.tensor.matmul(
                    poB[j][:], lhsT, xts[i][:, 512:1024],
                    start=(i == 0), stop=(i == tch - 1),
                )

        rzs = []
        for j in range(SCH):
            rzj = scrp.tile([128, 1], F32, name=f"rz{j}", tag=f"rz{j}")
            nc.vector.reciprocal(rzj[:], zps[j][:, 0:1])
            rzs.append(rzj)

        # Normalization: each [128, 512] PSUM half is scaled by 1/Z in
        # two [128, 256] pieces on ScalarE and VectorE in parallel, so
        # bank-freeing (which gates j2/j3's trailing groups) is fast.
        obs = [outp.tile([128, E], F32, name=f"ob{j}", tag=f"ob{j % 2}") for j in range(4)]

        def norm(j, po, lo):
            nc.scalar.mul(obs[j][:, lo : lo + 256], po[:, 0:256], rzs[j][:])
            nc.vector.tensor_scalar_mul(
                obs[j][:, lo + 256 : lo + 512], po[:, 256:512], rzs[j][:]
            )
            dma_eng = nc.sync if (j + lo // 512) % 2 == 0 else nc.scalar
            dma_eng.dma_start(
                out_d[128 * j : 128 * (j + 1), lo : lo + 512],
                obs[j][:, lo : lo + 512],
            )

        for j in range(SCH):
            norm(j, poA[j], 0)
        for j in range(SCH):
            norm(j, poB[j], 512)


def _build(tch):
    nc = bacc.Bacc(
        "TRN2",
        target_bir_lowering=False,
        debug=False,
        num_devices=N_CORES,
    )
    MISC_W = _misc_layout(tch)[3]
    x_d = nc.dram_tensor("x", [128 * tch, E], F16, kind="ExternalInput").ap()
    w_d = nc.dram_tensor("w", [128, E], F16, kind="ExternalInput").ap()
    misc_d = nc.dram_tensor("misc", [128, MISC_W], F16, kind="ExternalInput").ap()
    out_d = nc.dram_tensor("out", [SQ, E], F32, kind="ExternalOutput").ap()
    with tile.TileContext(nc) as tc:
        _build_body(tc, tch, out_d, x_d, w_d, misc_d)
    nc.compile()
    return nc


_NC_CACHE = {}


def _get_nc(tch):
    if tch not in _NC_CACHE:
        _NC_CACHE[tch] = _build(tch)
    return _NC_CACHE[tch]


def _make_in_maps(tch, x, W, b, start, end, groups, los):
    """groups[g] = span indices for group g; los[g] = first token of
    g's x window. Each group has exactly SQ spans whose tokens fit in
    [los[g], los[g] + 128*tch)."""
    x = np.asarray(x, dtype=np.float32)
    start = np.asarray(start, dtype=np.int32)
    end = np.asarray(end, dtype=np.int32)
    w16 = np.ascontiguousarray(
        np.broadcast_to(
            np.asarray(W, np.float32).reshape(1, E).astype(np.float16), (128, E)
        )
    )
    b_f32 = np.asarray(b, np.float32).reshape(1)
    nrow = 128 * tch
    in_maps = []
    for core in range(N_CORES):
        bb_idx, g = divmod(core, 4)
        idx = groups[g]
        lo = los[g]
        OFF_TCOL, OFF_STB, OFF_ENB, MISC_W = _misc_layout(tch)
        tcolv = (
            float(lo)
            + np.arange(128, dtype=np.float32)[:, None]
            + 128.0 * np.arange(tch, dtype=np.float32)[None, :]
        ).astype(np.float32)
        misc = np.empty((128, MISC_W), np.float16)
        misc[:, 0:2] = b_f32.view(np.float16)[None, :]
        misc[:, OFF_TCOL:OFF_STB] = tcolv.view(np.float16)
        misc[:, OFF_STB:OFF_ENB] = start[idx].astype(np.float16)[None, :]
        misc[:, OFF_ENB:MISC_W] = end[idx].astype(np.float16)[None, :]
        xw = np.zeros((nrow, E), np.float16)
        hi = min(lo + nrow, T)
        xw[: hi - lo] = x[bb_idx, lo:hi].astype(np.float16)
        in_maps.append(
            {
                "x": np.ascontiguousarray(xw),
                "w": w16,
                "misc": np.ascontiguousarray(misc),
            }
        )
    return in_maps


def run(x, W, b, start, end, trace=False, trace_cores=None):
    """Run on 8 cores; returns (out[B,S,E] f32, BassKernelResults)."""
    start_np = np.asarray(start, dtype=np.int32)
    end_np = np.asarray(end, dtype=np.int32)

    # Windowed sharding: sort spans by start, take quarters of 512. Use
    # the K=256 kernel iff every quarter's token span fits 256 rows.
    order = np.argsort(start_np, kind="stable")
    groups = [order[g * SQ : (g + 1) * SQ] for g in range(4)]
    los, ok = [], True
    for idx in groups:
        lo = int(start_np[idx].min())
        hi = int(end_np[idx].max())
        if hi - lo + 1 > 256:
            ok = False
            break
        los.append(min(lo, T - 1))
    if ok:
        tch = 2
    else:
        tch = 4
        groups = [np.arange(g * SQ, (g + 1) * SQ) for g in range(4)]
        los = [0, 0, 0, 0]

    nc = _get_nc(tch)
    in_maps = _make_in_maps(tch, x, W, b, start, end, groups, los)
    res = bass_utils.run_bass_kernel_spmd(
        nc,
        in_maps,
        core_ids=list(range(N_CORES)),
        trace=trace,
        trace_cores=trace_cores,
    )
    out = np.empty((B, S, E), np.float32)
    for core in range(N_CORES):
        bb_idx, g = divmod(core, 4)
        out[bb_idx, groups[g]] = res.results[core]["out"]
    return out, res


def kernel(x, W, b, start, end):
    out, _ = run(x, W, b, start, end, trace=False)
    return out
